# revision 1
# baseline (speedup 1.0000x reference)
"""KeOps-style multi-head attention (unnormalized-exp softmax) on 8 trn2 cores.

Sharding: core c handles batch bi = c//2 and query rows u*1024..(u+1)*1024
(u = c%2), ALL 8 heads. Output is a pure concat over cores (no reduction).

Per-core pipeline (one uniform SPMD program):
  A) DMA x (full batch rows for K/V, this core's rows for Q) + weights;
     transpose x on PE (128x128 identity-matmul transposes).
  B) QKV projections producing q^T/k^T in "stacked head" layout
     [32*h_local + d, n] (heads packed 4-per-tensor so the K=32 scores
     matmuls can be row-tiled 4x on the PE array), and v in normal layout
     with a ones-column appended (gives the softmax denominator for free
     from the same matmul that computes the numerator).
  C) Attention: scores^T chunks [nk=128, nq=1024] via 4x row-tiled K=32
     matmuls -> exp on ACT (psum->sbuf, [128,1024] per instr, the
     bottleneck engine) -> numer^T accumulation matmuls with e^T as the
     *moving* operand (K=128 full rate).
  D) Normalize by 1/(denom+eps) (DVE) and project with Wout + bias.
"""

import numpy as np
from contextlib import ExitStack

import concourse.bass as bass
import concourse.mybir as mybir
import concourse.tile as tile
from concourse import bacc
from concourse.bass_utils import run_bass_kernel_spmd
from concourse.masks import make_identity

DIM = 256
NUM_HEADS = 8
HEAD_DIM = 32
B = 4
N = 2048
NQ = 1024          # query rows per core
NCORES = 8
FP = mybir.dt.float32
EXP = mybir.ActivationFunctionType.Exp

NT_KV = N // 128   # 16 n-tiles of kv rows
NT_Q = NQ // 128   # 8 n-tiles of q rows
NGQ = NQ // 512    # 2 groups of 512 query cols in q^T
NGK = N // 512     # 4 groups in k^T free dim
NJ = N // 128      # 16 key chunks of 128


def build_program():
    nc = bacc.Bacc()

    xq = nc.declare_dram_parameter("xq", [NQ, DIM], FP, isOutput=False)
    xkv = nc.declare_dram_parameter("xkv", [N, DIM], FP, isOutput=False)
    wq = nc.declare_dram_parameter("wq", [DIM, DIM], FP, isOutput=False)
    wk = nc.declare_dram_parameter("wk", [DIM, DIM], FP, isOutput=False)
    wv = nc.declare_dram_parameter("wv", [DIM, DIM], FP, isOutput=False)
    wout = nc.declare_dram_parameter("wout", [DIM, DIM], FP, isOutput=False)
    bout = nc.declare_dram_parameter("bout", [DIM], FP, isOutput=False)
    ident_in = nc.declare_dram_parameter("ident", [128, 128], FP, isOutput=False)
    out = nc.declare_dram_parameter("out", [NQ, DIM], FP, isOutput=True)

    with tile.TileContext(nc) as tc, ExitStack() as ctx:
        consts = ctx.enter_context(tc.tile_pool(name="consts", bufs=1))
        persist = ctx.enter_context(tc.tile_pool(name="persist", bufs=1))

        ident = consts.tile([128, 128], FP)
        nc.sync.dma_start(out=ident, in_=ident_in[:, :])
        bias_b = consts.tile([128, DIM], FP)
        nc.sync.dma_start(out=bias_b, in_=bout[:].unsqueeze(0).to_broadcast([128, DIM]))

        # ---- weights ----
        # wq_sb/wk_sb/wv_sb: [128(c_local), ck, m]; lhsT slices are [128, 128]
        wq_sb = consts.tile([128, 2, DIM], FP)
        wk_sb = consts.tile([128, 2, DIM], FP)
        wv_sb = consts.tile([128, 2, DIM], FP)
        wout_sb = consts.tile([128, 2, DIM], FP)
        for ck in range(2):
            nc.sync.dma_start(out=wq_sb[:, ck, :], in_=wq[128 * ck:128 * (ck + 1), :])
            nc.sync.dma_start(out=wk_sb[:, ck, :], in_=wk[128 * ck:128 * (ck + 1), :])
            nc.sync.dma_start(out=wv_sb[:, ck, :], in_=wv[128 * ck:128 * (ck + 1), :])
            nc.sync.dma_start(out=wout_sb[:, ck, :], in_=wout[128 * ck:128 * (ck + 1), :])

        # ---- x loads (per 128-row tile so transposes can start early) ----
        xkv_sb = persist.tile([128, NT_KV, DIM], FP)
        for t in range(NT_KV):
            nc.sync.dma_start(out=xkv_sb[:, t, :], in_=xkv[128 * t:128 * (t + 1), :])
        xq_sb = persist.tile([128, NT_Q, DIM], FP)
        for t in range(NT_Q):
            nc.sync.dma_start(out=xq_sb[:, t, :], in_=xq[128 * t:128 * (t + 1), :])

        # ---- transposes: xkvT [128(c_local), ck, n], xqT [128, ck, nq] ----
        xkvT = persist.tile([128, 2, N], FP)
        xqT = persist.tile([128, 2, NQ], FP)
        with tc.tile_pool(name="tps", bufs=4, space="PSUM") as tps:
            for ck in range(2):
                for t in range(NT_KV):
                    ps = tps.tile([128, 128], FP)
                    nc.tensor.transpose(ps, xkv_sb[:, t, 128 * ck:128 * (ck + 1)], ident)
                    nc.vector.tensor_copy(xkvT[:, ck, 128 * t:128 * (t + 1)], ps)
                for t in range(NT_Q):
                    ps = tps.tile([128, 128], FP)
                    nc.tensor.transpose(ps, xq_sb[:, t, 128 * ck:128 * (ck + 1)], ident)
                    nc.vector.tensor_copy(xqT[:, ck, 128 * t:128 * (t + 1)], ps)

        # ---- QKV projections ----
        # qT/kT stacked-head layout: tensor i in {0,1} holds heads 4i..4i+3:
        # row 32*hloc + d  <->  head 4i+hloc, dim d.
        qT = [persist.tile([128, NQ], FP, tag=f"qT{i}", name=f"qT{i}") for i in range(2)]
        kT = [persist.tile([128, N], FP, tag=f"kT{i}", name=f"kT{i}") for i in range(2)]
        # v normal layout + ones column: [128(n), t, h, 33]
        v_sb = persist.tile([128, NT_KV, NUM_HEADS, HEAD_DIM + 1], FP)
        nc.vector.memset(v_sb[:, :, :, HEAD_DIM:], 1.0)

        with tc.tile_pool(name="qkvp", bufs=4, space="PSUM") as qkvp:
            for i in range(2):
                for g in range(NGQ):
                    ps = qkvp.tile([128, 512], FP, tag="proj")
                    for ck in range(2):
                        nc.tensor.matmul(
                            ps, lhsT=wq_sb[:, ck, 128 * i:128 * (i + 1)],
                            rhs=xqT[:, ck, 512 * g:512 * (g + 1)],
                            start=(ck == 0), stop=(ck == 1))
                    nc.vector.tensor_copy(qT[i][:, 512 * g:512 * (g + 1)], ps)
                for g in range(NGK):
                    ps = qkvp.tile([128, 512], FP, tag="proj")
                    for ck in range(2):
                        nc.tensor.matmul(
                            ps, lhsT=wk_sb[:, ck, 128 * i:128 * (i + 1)],
                            rhs=xkvT[:, ck, 512 * g:512 * (g + 1)],
                            start=(ck == 0), stop=(ck == 1))
                    nc.vector.tensor_copy(kT[i][:, 512 * g:512 * (g + 1)], ps)
            for t in range(NT_KV):
                ps = qkvp.tile([128, DIM], FP, tag="vproj")
                for ck in range(2):
                    nc.tensor.matmul(
                        ps, lhsT=xkvT[:, ck, 128 * t:128 * (t + 1)],
                        rhs=wv_sb[:, ck, :],
                        start=(ck == 0), stop=(ck == 1))
                # strided copy into the 33-wide per-head slots
                nc.vector.tensor_copy(v_sb[:, t, :, 0:HEAD_DIM], ps)

        # ---- attention ----
        # PT: normalized pre-projection, transposed: tensor i rows = wout rows
        # 128i..128i+128 (head 4i+hloc dim d at partition 32*hloc+d).
        PT = [persist.tile([128, NQ], FP, tag=f"PT{i}", name=f"PT{i}") for i in range(2)]
        denom = persist.tile([16, 512], FP)   # row 8*g + h
        recip = persist.tile([16, 512], FP)

        with (
            tc.tile_pool(name="spsum", bufs=2, space="PSUM") as spsum,
            tc.tile_pool(name="npsum", bufs=1, space="PSUM") as npsum,
            tc.tile_pool(name="esb", bufs=3) as esb,
            tc.tile_pool(name="evac", bufs=4) as evac,
        ):
            for g in range(NGQ):
                for hh in range(2):
                    nps = [npsum.tile([HEAD_DIM + 1, 512], FP, tag=f"np{x}", name=f"np{x}")
                           for x in range(4)]
                    for j in range(NJ):
                        for p in range(2):
                            sp = spsum.tile([128, 1024], FP, tag="sp")
                            for uu in range(2):
                                hloc = 2 * p + uu
                                r = 32 * hloc
                                nc.tensor.matmul(
                                    sp[:, 512 * uu:512 * (uu + 1)],
                                    lhsT=kT[hh][r:r + 32, 128 * j:128 * (j + 1)],
                                    rhs=qT[hh][r:r + 32, 512 * g:512 * (g + 1)],
                                    start=True, stop=True,
                                    tile_position=(r, 0))
                            e = esb.tile([128, 1024], FP, tag="e")
                            nc.scalar.activation(e, sp, EXP)
                            for uu in range(2):
                                hloc = 2 * p + uu
                                h = 4 * hh + hloc
                                nc.tensor.matmul(
                                    nps[hloc],
                                    lhsT=v_sb[:, j, h, :],
                                    rhs=e[:, 512 * uu:512 * (uu + 1)],
                                    start=(j == 0), stop=(j == NJ - 1))
                    for hloc in range(4):
                        tmp = evac.tile([HEAD_DIM + 1, 512], FP, tag="ev")
                        nc.vector.tensor_copy(tmp, nps[hloc])
                        nc.sync.dma_start(
                            out=PT[hh][32 * hloc:32 * hloc + 32,
                                       512 * g:512 * (g + 1)],
                            in_=tmp[0:HEAD_DIM, :])
                        r = 8 * g + 4 * hh + hloc
                        nc.sync.dma_start(out=denom[r:r + 1, :],
                                          in_=tmp[HEAD_DIM:HEAD_DIM + 1, :])

            # denominators -> reciprocals -> broadcast -> normalize
            nc.vector.tensor_scalar_add(denom, denom, 1e-6)
            nc.vector.reciprocal(recip, denom)
            # broadcast recip rows across partitions via a DRAM bounce
            # (DMA partition-broadcast is only legal from DRAM sources)
            with tc.tile_pool(name="dscratch", bufs=1, space="DRAM") as dsc:
                recip_dram = dsc.tile([16, 512], FP)
                nc.sync.dma_start(out=recip_dram[:, :], in_=recip)
                rb = [persist.tile([128, NQ], FP, tag=f"rb{i}", name=f"rb{i}") for i in range(2)]
                for g in range(NGQ):
                    for hh in range(2):
                        for hloc in range(4):
                            r = 8 * g + 4 * hh + hloc
                            nc.sync.dma_start(
                                out=rb[hh][32 * hloc:32 * hloc + 32,
                                           512 * g:512 * (g + 1)],
                                in_=recip_dram[r:r + 1, :].to_broadcast([32, 512]))
            for i in range(2):
                nc.vector.tensor_mul(PT[i], PT[i], rb[i])

        # ---- output projection ----
        with (
            tc.tile_pool(name="opsum", bufs=4, space="PSUM") as opsum,
            tc.tile_pool(name="osb", bufs=4) as osb,
        ):
            for t in range(NT_Q):
                ps = opsum.tile([128, DIM], FP, tag="o")
                for i in range(2):
                    nc.tensor.matmul(
                        ps, lhsT=PT[i][:, 128 * t:128 * (t + 1)],
                        rhs=wout_sb[:, i, :],
                        start=(i == 0), stop=(i == 1))
                ob = osb.tile([128, DIM], FP, tag="ob")
                nc.vector.tensor_add(ob, ps, bias_b)
                nc.sync.dma_start(out=out[128 * t:128 * (t + 1), :], in_=ob)

    if not nc.is_finalized():
        nc.finalize()
    return nc


_NC_CACHE = None


def _get_program():
    global _NC_CACHE
    if _NC_CACHE is None:
        _NC_CACHE = build_program()
    return _NC_CACHE


def kernel(x, Wqkv, Wout, bout, _trace=False, _trace_kwargs=None):
    x = np.asarray(x, dtype=np.float32)
    Wqkv = np.asarray(Wqkv, dtype=np.float32)
    Wout = np.asarray(Wout, dtype=np.float32)
    bout = np.asarray(bout, dtype=np.float32)

    scale = HEAD_DIM ** -0.5
    wq = np.ascontiguousarray(Wqkv[:, 0:DIM] * scale)
    wk = np.ascontiguousarray(Wqkv[:, DIM:2 * DIM])
    wv = np.ascontiguousarray(Wqkv[:, 2 * DIM:3 * DIM])

    in_maps = []
    for c in range(NCORES):
        bi, u = c // 2, c % 2
        in_maps.append({
            "xq": np.ascontiguousarray(x[bi, u * NQ:(u + 1) * NQ, :]),
            "xkv": np.ascontiguousarray(x[bi]),
            "wq": wq, "wk": wk, "wv": wv,
            "wout": np.ascontiguousarray(Wout),
            "bout": bout,
            "ident": np.eye(128, dtype=np.float32),
        })

    nc = _get_program()
    kwargs = {}
    if _trace:
        kwargs["trace"] = True
        if _trace_kwargs:
            kwargs.update(_trace_kwargs)
    res = run_bass_kernel_spmd(nc, in_maps, core_ids=list(range(NCORES)), **kwargs)

    outf = np.empty((B, N, DIM), dtype=np.float32)
    for c in range(NCORES):
        bi, u = c // 2, c % 2
        outf[bi, u * NQ:(u + 1) * NQ, :] = res.results[c]["out"]
    if _trace:
        return outf, res
    return outf



# revision 11
# speedup vs baseline: 1.7593x; 1.7593x over previous
"""KeOps-style multi-head attention (unnormalized-exp softmax) on 8 trn2 cores.

Sharding: core c handles batch bi = c//2 and query rows u*1024..(u+1)*1024
(u = c%2), ALL 8 heads. Output is a pure concat over cores (no reduction).

All matmuls run in bf16 (1 col/cycle on the PE, FWL-eligible weight loads;
fp32/fp32r matmuls are 4x slower and poison FWL for the next matmul).
Host pre-casts x and the weights to bf16, halving input DMA. PSUM
accumulation stays fp32, exp input is the fp32 psum scores, and the
softmax ratio cancels most of the bf16 rounding.

Per-core pipeline (one uniform SPMD program):
  A) DMA bf16 x (full batch rows for K/V, this core's rows for Q) +
     bf16 weights; transpose x on PE (128x128 identity-matmul transposes).
  B) QKV projections producing q^T/k^T in "stacked head" layout
     [32*h_local + d, n] (heads packed 4-per-tensor so the K=32 scores
     matmuls can be row-tiled 4x on the PE array), and v in normal layout
     with a ones-column appended (gives the softmax denominator for free
     from the same matmul that computes the numerator).
  C) Attention: scores^T chunks [nk=128, nq=1024] via 4x row-tiled K=32
     matmuls -> exp on ACT (psum->sbuf bf16, [128,1024] per instr) ->
     numer^T accumulation matmuls with e^T as the *moving* operand.
  D) Normalize by 1/(denom+eps) (DVE, writes bf16) and project with
     Wout + bias (fp32 out).
"""

import numpy as np
import ml_dtypes
from contextlib import ExitStack

import concourse.bass as bass
import concourse.mybir as mybir
import concourse.tile as tile
from concourse import bacc
from concourse.bass_utils import run_bass_kernel_spmd

DIM = 256
NUM_HEADS = 8
HEAD_DIM = 32
B = 4
N = 2048
NQ = 1024          # query rows per core
NCORES = 8
FP = mybir.dt.float32
BF = mybir.dt.bfloat16
EXP = mybir.ActivationFunctionType.Exp

NT_KV = N // 128   # 16 n-tiles of kv rows
NT_Q = NQ // 128   # 8 n-tiles of q rows
NGQ = NQ // 512    # 2 groups of 512 query cols in q^T
NGK = N // 512     # 4 groups in k^T free dim
NJ = N // 128      # 16 key chunks of 128


def build_program():
    nc = bacc.Bacc()

    xq = nc.declare_dram_parameter("xq", [NQ, DIM], BF, isOutput=False)
    xkv = nc.declare_dram_parameter("xkv", [N, DIM], BF, isOutput=False)
    wq = nc.declare_dram_parameter("wq", [DIM, DIM], BF, isOutput=False)
    wk = nc.declare_dram_parameter("wk", [DIM, DIM], BF, isOutput=False)
    wv = nc.declare_dram_parameter("wv", [DIM, DIM], BF, isOutput=False)
    wout = nc.declare_dram_parameter("wout", [DIM, DIM], BF, isOutput=False)
    bout = nc.declare_dram_parameter("bout", [DIM], FP, isOutput=False)
    ident_in = nc.declare_dram_parameter("ident", [128, 128], BF, isOutput=False)
    out = nc.declare_dram_parameter("out", [NQ, DIM], FP, isOutput=True)

    with tile.TileContext(nc) as tc, ExitStack() as ctx:
        consts = ctx.enter_context(tc.tile_pool(name="consts", bufs=1))
        persist = ctx.enter_context(tc.tile_pool(name="persist", bufs=1))

        ident = consts.tile([128, 128], BF)
        nc.sync.dma_start(out=ident, in_=ident_in[:, :])
        bias_b = consts.tile([128, DIM], FP)
        nc.sync.dma_start(out=bias_b, in_=bout[:].unsqueeze(0).to_broadcast([128, DIM]))

        # ---- weights ----
        # wq_sb/wk_sb/wv_sb: [128(c_local), ck, m]; lhsT slices are [128, 128]
        wq_sb = consts.tile([128, 2, DIM], BF)
        wk_sb = consts.tile([128, 2, DIM], BF)
        wv_sb = consts.tile([128, 2, DIM], BF)
        wout_sb = consts.tile([128, 2, DIM], BF)
        for ck in range(2):
            nc.sync.dma_start(out=wq_sb[:, ck, :], in_=wq[128 * ck:128 * (ck + 1), :])
            nc.sync.dma_start(out=wk_sb[:, ck, :], in_=wk[128 * ck:128 * (ck + 1), :])
            nc.sync.dma_start(out=wv_sb[:, ck, :], in_=wv[128 * ck:128 * (ck + 1), :])
            nc.sync.dma_start(out=wout_sb[:, ck, :], in_=wout[128 * ck:128 * (ck + 1), :])

        # ---- x loads (per 128-row tile so transposes can start early) ----
        xkv_sb = persist.tile([128, NT_KV, DIM], BF)
        for t in range(NT_KV):
            nc.sync.dma_start(out=xkv_sb[:, t, :], in_=xkv[128 * t:128 * (t + 1), :])
        xq_sb = persist.tile([128, NT_Q, DIM], BF)
        for t in range(NT_Q):
            nc.sync.dma_start(out=xq_sb[:, t, :], in_=xq[128 * t:128 * (t + 1), :])

        # ---- transposes: xkvT [128(c_local), ck, n], xqT [128, ck, nq] ----
        xkvT = persist.tile([128, 2, N], BF)
        xqT = persist.tile([128, 2, NQ], BF)
        with tc.tile_pool(name="tps", bufs=4, space="PSUM") as tps:
            for ck in range(2):
                for t in range(NT_KV):
                    ps = tps.tile([128, 128], BF)
                    nc.tensor.transpose(ps, xkv_sb[:, t, 128 * ck:128 * (ck + 1)], ident)
                    nc.vector.tensor_copy(xkvT[:, ck, 128 * t:128 * (t + 1)], ps)
                for t in range(NT_Q):
                    ps = tps.tile([128, 128], BF)
                    nc.tensor.transpose(ps, xq_sb[:, t, 128 * ck:128 * (ck + 1)], ident)
                    nc.vector.tensor_copy(xqT[:, ck, 128 * t:128 * (t + 1)], ps)

        # ---- QKV projections ----
        # qT/kT stacked-head layout: tensor i in {0,1} holds heads 4i..4i+3:
        # row 32*hloc + d  <->  head 4i+hloc, dim d.
        qT = [persist.tile([128, NQ], BF, tag=f"qT{i}", name=f"qT{i}") for i in range(2)]
        kT = [persist.tile([128, N], BF, tag=f"kT{i}", name=f"kT{i}") for i in range(2)]
        # v normal layout + ones column: [128(n), t, h, 33] (the ones column
        # is exact in bf16 and yields the denominator from the numer matmul)
        v_sb = persist.tile([128, NT_KV, NUM_HEADS, HEAD_DIM + 1], BF)
        nc.vector.memset(v_sb[:, :, :, HEAD_DIM:], 1.0)

        with tc.tile_pool(name="qkvp", bufs=4, space="PSUM") as qkvp:
            for i in range(2):
                for g in range(NGQ):
                    ps = qkvp.tile([128, 512], FP, tag="proj")
                    for ck in range(2):
                        nc.tensor.matmul(
                            ps, lhsT=wq_sb[:, ck, 128 * i:128 * (i + 1)],
                            rhs=xqT[:, ck, 512 * g:512 * (g + 1)],
                            start=(ck == 0), stop=(ck == 1))
                    nc.vector.tensor_copy(qT[i][:, 512 * g:512 * (g + 1)], ps)
                for g in range(NGK):
                    ps = qkvp.tile([128, 512], FP, tag="proj")
                    for ck in range(2):
                        nc.tensor.matmul(
                            ps, lhsT=wk_sb[:, ck, 128 * i:128 * (i + 1)],
                            rhs=xkvT[:, ck, 512 * g:512 * (g + 1)],
                            start=(ck == 0), stop=(ck == 1))
                    nc.vector.tensor_copy(kT[i][:, 512 * g:512 * (g + 1)], ps)
            for t in range(NT_KV):
                ps = qkvp.tile([128, DIM], FP, tag="vproj")
                for ck in range(2):
                    nc.tensor.matmul(
                        ps, lhsT=xkvT[:, ck, 128 * t:128 * (t + 1)],
                        rhs=wv_sb[:, ck, :],
                        start=(ck == 0), stop=(ck == 1))
                # strided copy into the 33-wide per-head slots
                nc.vector.tensor_copy(v_sb[:, t, :, 0:HEAD_DIM], ps)

        # ---- attention ----
        # PTraw: unnormalized numer, transposed: tensor i rows = wout rows
        # 128i..128i+128 (head 4i+hloc dim d at partition 32*hloc+d).
        PTraw = [persist.tile([128, NQ], FP, tag=f"PTr{i}", name=f"PTr{i}") for i in range(2)]
        PTb = [persist.tile([128, NQ], BF, tag=f"PTb{i}", name=f"PTb{i}") for i in range(2)]
        denom = persist.tile([16, 512], FP)   # row 8*g + h
        recip = persist.tile([16, 512], FP)

        with (
            tc.tile_pool(name="spsum", bufs=2, space="PSUM") as spsum,
            tc.tile_pool(name="npsum", bufs=1, space="PSUM") as npsum,
            tc.tile_pool(name="esb", bufs=3) as esb,
            tc.tile_pool(name="evac", bufs=4) as evac,
        ):
            for g in range(NGQ):
                for hh in range(2):
                    nps = [npsum.tile([HEAD_DIM + 1, 512], FP, tag=f"np{x}", name=f"np{x}")
                           for x in range(4)]
                    for j in range(NJ):
                        for p in range(2):
                            sp = spsum.tile([128, 1024], FP, tag="sp")
                            for uu in range(2):
                                hloc = 2 * p + uu
                                r = 32 * hloc
                                nc.tensor.matmul(
                                    sp[:, 512 * uu:512 * (uu + 1)],
                                    lhsT=kT[hh][r:r + 32, 128 * j:128 * (j + 1)],
                                    rhs=qT[hh][r:r + 32, 512 * g:512 * (g + 1)],
                                    start=True, stop=True,
                                    tile_position=(r, 0))
                            e = esb.tile([128, 1024], BF, tag="e")
                            nc.scalar.activation(e, sp, EXP)
                            for uu in range(2):
                                hloc = 2 * p + uu
                                h = 4 * hh + hloc
                                nc.tensor.matmul(
                                    nps[hloc],
                                    lhsT=v_sb[:, j, h, :],
                                    rhs=e[:, 512 * uu:512 * (uu + 1)],
                                    start=(j == 0), stop=(j == NJ - 1))
                    for hloc in range(4):
                        tmp = evac.tile([HEAD_DIM + 1, 512], FP, tag="ev")
                        nc.vector.tensor_copy(tmp, nps[hloc])
                        nc.sync.dma_start(
                            out=PTraw[hh][32 * hloc:32 * hloc + 32,
                                          512 * g:512 * (g + 1)],
                            in_=tmp[0:HEAD_DIM, :])
                        r = 8 * g + 4 * hh + hloc
                        nc.sync.dma_start(out=denom[r:r + 1, :],
                                          in_=tmp[HEAD_DIM:HEAD_DIM + 1, :])

            # denominators -> reciprocals -> broadcast -> normalize
            nc.vector.tensor_scalar_add(denom, denom, 1e-6)
            nc.vector.reciprocal(recip, denom)
            # broadcast recip rows across partitions via a DRAM bounce
            # (DMA partition-broadcast is only legal from DRAM sources)
            with tc.tile_pool(name="dscratch", bufs=1, space="DRAM") as dsc:
                recip_dram = dsc.tile([16, 512], FP)
                nc.sync.dma_start(out=recip_dram[:, :], in_=recip)
                rb = [persist.tile([128, NQ], FP, tag=f"rb{i}", name=f"rb{i}") for i in range(2)]
                for g in range(NGQ):
                    for hh in range(2):
                        for hloc in range(4):
                            r = 8 * g + 4 * hh + hloc
                            nc.sync.dma_start(
                                out=rb[hh][32 * hloc:32 * hloc + 32,
                                           512 * g:512 * (g + 1)],
                                in_=recip_dram[r:r + 1, :].to_broadcast([32, 512]))
            for i in range(2):
                nc.vector.tensor_mul(PTb[i], PTraw[i], rb[i])

        # ---- output projection ----
        with (
            tc.tile_pool(name="opsum", bufs=4, space="PSUM") as opsum,
            tc.tile_pool(name="osb", bufs=4) as osb,
        ):
            for t in range(NT_Q):
                ps = opsum.tile([128, DIM], FP, tag="o")
                for i in range(2):
                    nc.tensor.matmul(
                        ps, lhsT=PTb[i][:, 128 * t:128 * (t + 1)],
                        rhs=wout_sb[:, i, :],
                        start=(i == 0), stop=(i == 1))
                ob = osb.tile([128, DIM], FP, tag="ob")
                nc.vector.tensor_add(ob, ps, bias_b)
                nc.sync.dma_start(out=out[128 * t:128 * (t + 1), :], in_=ob)

    if not nc.is_finalized():
        nc.finalize()
    return nc


_NC_CACHE = None


def _get_program():
    global _NC_CACHE
    if _NC_CACHE is None:
        _NC_CACHE = build_program()
    return _NC_CACHE


def kernel(x, Wqkv, Wout, bout, _trace=False, _trace_kwargs=None):
    x = np.asarray(x, dtype=np.float32)
    Wqkv = np.asarray(Wqkv, dtype=np.float32)
    Wout = np.asarray(Wout, dtype=np.float32)
    bout = np.asarray(bout, dtype=np.float32)

    bf = ml_dtypes.bfloat16
    scale = HEAD_DIM ** -0.5
    wq = np.ascontiguousarray((Wqkv[:, 0:DIM] * scale).astype(bf))
    wk = np.ascontiguousarray(Wqkv[:, DIM:2 * DIM].astype(bf))
    wv = np.ascontiguousarray(Wqkv[:, 2 * DIM:3 * DIM].astype(bf))
    wout_bf = np.ascontiguousarray(Wout.astype(bf))
    x_bf = x.astype(bf)

    in_maps = []
    for c in range(NCORES):
        bi, u = c // 2, c % 2
        in_maps.append({
            "xq": np.ascontiguousarray(x_bf[bi, u * NQ:(u + 1) * NQ, :]),
            "xkv": np.ascontiguousarray(x_bf[bi]),
            "wq": wq, "wk": wk, "wv": wv,
            "wout": wout_bf,
            "bout": bout,
            "ident": np.eye(128, dtype=bf),
        })

    nc = _get_program()
    kwargs = {}
    if _trace:
        kwargs["trace"] = True
        if _trace_kwargs:
            kwargs.update(_trace_kwargs)
    res = run_bass_kernel_spmd(nc, in_maps, core_ids=list(range(NCORES)), **kwargs)

    outf = np.empty((B, N, DIM), dtype=np.float32)
    for c in range(NCORES):
        bi, u = c // 2, c % 2
        outf[bi, u * NQ:(u + 1) * NQ, :] = res.results[c]["out"]
    if _trace:
        return outf, res
    return outf


# revision 24
# speedup vs baseline: 2.2794x; 1.2956x over previous
"""KeOps-style multi-head attention (unnormalized-exp softmax) on 8 trn2 cores.

Sharding: core c handles batch bi = c//2 and query rows u*1024..(u+1)*1024
(u = c%2), ALL 8 heads. Output is a pure concat over cores (no reduction).

Engine budget (per core, under the sustained-load PE clock throttle to
~1.2 GHz): the ACT engine's 128 exp instructions ([128,1024] each,
~1.2us) are the ~154us spine; everything else is arranged to hide under
it. Scores matmuls run in bf16 (1 col/cycle, row-tiled pairs overlap),
the numerator runs as fp8e4 DoubleRow over 256-key pairs (2 MACs/cycle),
and exp is written as fp8 e' = exp(s - CBIAS) (the softmax ratio is
exactly invariant to the shift; CBIAS keeps e' inside fp8e4's max 240).

Program order matters because engines execute in-order: the attention
blocks are emitted as soon as kT[0]/v2/qT[0] exist, and the remaining
projections, the per-(g,hh) denom->recip->normalize chains, and the
output projection are interleaved between blocks so they overlap the
ACT-bound loop. x arrives pre-transposed via DMA-transpose loads.
"""

import numpy as np
import ml_dtypes
from contextlib import ExitStack

import concourse.bass as bass
import concourse.mybir as mybir
import concourse.tile as tile
from concourse import bacc
from concourse.bass_utils import run_bass_kernel_spmd

DIM = 256
NUM_HEADS = 8
HEAD_DIM = 32
B = 4
N = 2048
NQ = 1024          # query rows per core
NCORES = 8
FP = mybir.dt.float32
BF = mybir.dt.bfloat16
FP8 = mybir.dt.float8e4
EXP = mybir.ActivationFunctionType.Exp
CBIAS = 3.8   # exp(s - CBIAS): this stack's float8e4 is IEEE e4m3 (max finite
              # 240); smax~8.6 plus bf16 jitter -> e^4.9=134 < 240

NT_KV = N // 128   # 16 n-tiles of kv rows
NGQ = NQ // 512    # 2 groups of 512 query cols in q^T
NGK = N // 512     # 4 groups in k^T free dim
NJ = N // 128      # 16 key chunks of 128
NT_Q = NQ // 128   # 8 output row tiles


def build_program():
    nc = bacc.Bacc()

    xq = nc.declare_dram_parameter("xq", [NQ, DIM], BF, isOutput=False)
    xkv = nc.declare_dram_parameter("xkv", [N, DIM], BF, isOutput=False)
    wq = nc.declare_dram_parameter("wq", [DIM, DIM], BF, isOutput=False)
    wk = nc.declare_dram_parameter("wk", [DIM, DIM], BF, isOutput=False)
    wv = nc.declare_dram_parameter("wv", [DIM, DIM], BF, isOutput=False)
    wout = nc.declare_dram_parameter("wout", [DIM, DIM], BF, isOutput=False)
    bout = nc.declare_dram_parameter("bout", [DIM], FP, isOutput=False)
    out = nc.declare_dram_parameter("out", [NQ, DIM], FP, isOutput=True)

    with tile.TileContext(nc) as tc, ExitStack() as ctx:
        consts = ctx.enter_context(tc.tile_pool(name="consts", bufs=1))
        persist = ctx.enter_context(tc.tile_pool(name="persist", bufs=1))

        # ---- transposed x via DMA-transpose (no PE transposes needed) ----
        # xkvT[c, ck, n] = xkv[n, 128*ck + c]; loaded in 512-row chunks so
        # the K/V projections can start before the whole tensor lands.
        xkvT = persist.tile([128, 2, N], BF)
        xqT = persist.tile([128, 2, NQ], BF)
        order = [("kv", 0), ("kv", 1), ("q", 0), ("kv", 2), ("kv", 3), ("q", 1)]
        for kind, c in order:
            if kind == "kv":
                nc.sync.dma_start_transpose(
                    out=xkvT[:, :, 512 * c:512 * (c + 1)],
                    in_=xkv[512 * c:512 * (c + 1), :])
            else:
                nc.sync.dma_start_transpose(
                    out=xqT[:, :, 512 * c:512 * (c + 1)],
                    in_=xq[512 * c:512 * (c + 1), :])

        # ---- weights / consts ----
        wq_sb = consts.tile([128, 2, DIM], BF)
        wk_sb = consts.tile([128, 2, DIM], BF)
        wv_sb = consts.tile([128, 2, DIM], BF)
        wout_sb = consts.tile([128, 2, DIM], BF)
        for ck in range(2):
            nc.gpsimd.dma_start(out=wk_sb[:, ck, :], in_=wk[128 * ck:128 * (ck + 1), :])
            nc.gpsimd.dma_start(out=wq_sb[:, ck, :], in_=wq[128 * ck:128 * (ck + 1), :])
        for ck in range(2):
            nc.gpsimd.dma_start(out=wv_sb[:, ck, :], in_=wv[128 * ck:128 * (ck + 1), :])
            nc.gpsimd.dma_start(out=wout_sb[:, ck, :], in_=wout[128 * ck:128 * (ck + 1), :])
        negC = consts.tile([128, 1], FP)
        nc.vector.memset(negC, -CBIAS)
        bias_b = consts.tile([128, DIM], FP)
        nc.gpsimd.dma_start(out=bias_b, in_=bout[:].unsqueeze(0).to_broadcast([128, DIM]))

        # ---- persistent attention state ----
        # qT/kT stacked-head layout: tensor i holds heads 4i..4i+3; row
        # 32*hloc + d <-> head 4i+hloc, dim d.
        qT = [persist.tile([128, NQ], BF, tag=f"qT{i}", name=f"qT{i}") for i in range(2)]
        kT = [persist.tile([128, N], BF, tag=f"kT{i}", name=f"kT{i}") for i in range(2)]
        # v fp8 layout for DoubleRow: [128(n), t(key-pair), s(chunk), h, 36];
        # col 32 is the ones column (denominator); 36-wide slots keep the
        # DoubleRow pair stride 16B-aligned (8*36=288).
        v2 = persist.tile([128, NJ // 2, 2, NUM_HEADS, 36], FP8)
        nc.vector.memset(v2[:, :, :, :, HEAD_DIM:HEAD_DIM + 1], 1.0)
        PTraw = [persist.tile([128, NQ], FP, tag=f"PTr{i}", name=f"PTr{i}") for i in range(2)]
        PTb = [persist.tile([128, NQ], BF, tag=f"PTb{i}", name=f"PTb{i}") for i in range(2)]
        rb = [persist.tile([128, NQ], FP, tag=f"rb{i}", name=f"rb{i}") for i in range(2)]
        # (g,hh) group gi=2g+hh lives at partition 32*gi (+hloc) so DVE ops
        # on a group's 4 rows start at a 32-aligned partition base
        denom = persist.tile([128, 512], FP)
        recip = persist.tile([128, 512], FP)
        # out-projection partials: obp[t] = PTb[0].T @ Wout[0:128] + bias,
        # accumulated early so the tail only runs the i=1 halves
        obp = [persist.tile([128, DIM], FP, tag=f"obp{t}", name=f"obp{t}")
               for t in range(NT_Q)]

        with (
            tc.tile_pool(name="spsum", bufs=3, space="PSUM") as spsum,
            tc.tile_pool(name="npsum", bufs=1, space="PSUM") as npsum,
            tc.tile_pool(name="esb", bufs=4) as esb,
            tc.tile_pool(name="evac", bufs=4) as evac,
            tc.tile_pool(name="osb", bufs=4) as osb,
            tc.tile_pool(name="dscratch", bufs=1, space="DRAM") as dsc,
        ):
            recip_dram = dsc.tile([16, 512], FP)

            def emit_qproj(i, g):
                ps = spsum.tile([128, 1024], FP, tag="sp", name="pjq")
                for ck in range(2):
                    nc.tensor.matmul(
                        ps[:, 0:512], lhsT=wq_sb[:, ck, 128 * i:128 * (i + 1)],
                        rhs=xqT[:, ck, 512 * g:512 * (g + 1)],
                        start=(ck == 0), stop=(ck == 1))
                nc.vector.tensor_copy(qT[i][:, 512 * g:512 * (g + 1)], ps[:, 0:512])

            def emit_kproj(i, gs=None):
                for g in (range(NGK) if gs is None else gs):
                    ps = spsum.tile([128, 1024], FP, tag="sp", name="pjk")
                    for ck in range(2):
                        nc.tensor.matmul(
                            ps[:, 0:512], lhsT=wk_sb[:, ck, 128 * i:128 * (i + 1)],
                            rhs=xkvT[:, ck, 512 * g:512 * (g + 1)],
                            start=(ck == 0), stop=(ck == 1))
                    nc.vector.tensor_copy(kT[i][:, 512 * g:512 * (g + 1)], ps[:, 0:512])

            def emit_vproj(ts):
                for t in ts:
                    ps = spsum.tile([128, 1024], FP, tag="sp", name="pjv")
                    for ck in range(2):
                        nc.tensor.matmul(
                            ps[:, 0:DIM], lhsT=xkvT[:, ck, 128 * t:128 * (t + 1)],
                            rhs=wv_sb[:, ck, :],
                            start=(ck == 0), stop=(ck == 1))
                    nc.vector.tensor_copy(v2[:, t // 2, t % 2, :, 0:HEAD_DIM],
                                          ps[:, 0:DIM])

            def emit_block(g, hh, pp, hooks=None):
                nps = [npsum.tile([HEAD_DIM + 1, 512], FP, tag=f"np{x}",
                                  name=f"np{x}") for x in range(2)]
                e2 = None
                for j in range(NJ):
                    if hooks and j in hooks:
                        for fn in hooks[j]:
                            fn()
                    sp = spsum.tile([128, 1024], FP, tag="sp", name="sp")
                    for uu in range(2):
                        hloc = 2 * pp + uu
                        r = 32 * hloc
                        nc.tensor.matmul(
                            sp[:, 512 * uu:512 * (uu + 1)],
                            lhsT=kT[hh][r:r + 32, 128 * j:128 * (j + 1)],
                            rhs=qT[hh][r:r + 32, 512 * g:512 * (g + 1)],
                            start=True, stop=True,
                            tile_position=(r, 0))
                    if j % 2 == 0:
                        e2 = esb.tile([128, 2, 1024], FP8, tag="e2", name="e2")
                    nc.scalar.activation(e2[:, j % 2, :], sp, EXP, bias=negC[:, 0:1])
                    if j % 2 == 1:
                        t = j // 2
                        for uu in range(2):
                            h = 4 * hh + 2 * pp + uu
                            nc.tensor.matmul(
                                nps[uu],
                                lhsT=v2[:, t, 0:2, h, 0:HEAD_DIM + 1],
                                rhs=e2[:, 0:2, 512 * uu:512 * (uu + 1)],
                                start=(t == 0), stop=(t == NJ // 2 - 1),
                                perf_mode=mybir.MatmulPerfMode.DoubleRow)
                for uu in range(2):
                    hloc = 2 * pp + uu
                    tmp = evac.tile([HEAD_DIM + 1, 512], FP, tag="ev", name="ev")
                    nc.vector.tensor_copy(tmp, nps[uu])
                    nc.sync.dma_start(
                        out=PTraw[hh][32 * hloc:32 * hloc + 32,
                                      512 * g:512 * (g + 1)],
                        in_=tmp[0:HEAD_DIM, :])
                    r = 32 * (2 * g + hh) + hloc
                    nc.sync.dma_start(out=denom[r:r + 1, :],
                                      in_=tmp[HEAD_DIM:HEAD_DIM + 1, :])

            def emit_norm(g, hh):
                # denom -> recip -> DRAM-bounce partition-broadcast ->
                # normalized bf16 PT for this (g, hh); runs off the PE/ACT
                r0 = 32 * (2 * g + hh)
                d0 = 4 * (2 * g + hh)
                # eps (1e-6) skipped: denom' = e^-C * sum(e^s) is ~1e2 here, so
                # the reference's +1e-6 changes nothing at fp32 resolution
                nc.vector.reciprocal(recip[r0:r0 + 4, :], denom[r0:r0 + 4, :])
                nc.sync.dma_start(out=recip_dram[d0:d0 + 4, :],
                                  in_=recip[r0:r0 + 4, :])
                for hloc in range(4):
                    nc.sync.dma_start(
                        out=rb[hh][32 * hloc:32 * hloc + 32,
                                   512 * g:512 * (g + 1)],
                        in_=recip_dram[d0 + hloc:d0 + hloc + 1, :].to_broadcast([32, 512]))
                nc.vector.tensor_mul(PTb[hh][:, 512 * g:512 * (g + 1)],
                                     PTraw[hh][:, 512 * g:512 * (g + 1)],
                                     rb[hh][:, 512 * g:512 * (g + 1)])

            def emit_oph(t, i):
                # one half of the output projection for row-tile t; i=0
                # stashes partial+bias in obp[t], i=1 completes and stores
                ps = spsum.tile([128, 1024], FP, tag="sp", name="spo")
                nc.tensor.matmul(
                    ps[:, 0:DIM], lhsT=PTb[i][:, 128 * t:128 * (t + 1)],
                    rhs=wout_sb[:, i, :], start=True, stop=True)
                if i == 0:
                    nc.vector.tensor_add(obp[t], ps[:, 0:DIM], bias_b)
                else:
                    ob = osb.tile([128, DIM], FP, tag="ob", name="ob")
                    nc.vector.tensor_add(ob, ps[:, 0:DIM], obp[t])
                    nc.sync.dma_start(out=out[128 * t:128 * (t + 1), :], in_=ob)

            # ---- minimal pre-loop projections ----
            emit_kproj(0)
            emit_qproj(0, 0)
            emit_vproj(range(NT_KV))

            # ---- attention blocks; leftover projections, normalize
            # chains, and out-projection halves sit between blocks where
            # their inputs are long since ready ----
            after = {
                (0, 0, 0): [lambda: emit_qproj(0, 1)],
                (0, 0, 1): [lambda: emit_kproj(1),
                            lambda: emit_norm(0, 0)],
                (0, 1, 0): [lambda: emit_qproj(1, 0)],
                (0, 1, 1): [lambda: emit_qproj(1, 1),
                            lambda: emit_norm(1, 0)],
                (1, 0, 0): [lambda t=t: emit_oph(t, 0) for t in range(0, 4)],
                (1, 0, 1): [lambda: emit_norm(0, 1)] +
                           [lambda t=t: emit_oph(t, 0) for t in range(4, NT_Q)],
                (1, 1, 0): [lambda t=t: emit_oph(t, 1) for t in range(0, 4)],
                (1, 1, 1): [lambda: emit_norm(1, 1)] +
                           [lambda t=t: emit_oph(t, 1) for t in range(4, NT_Q)],
            }
            for hh in range(2):
                for g in range(NGQ):
                    for pp in range(2):
                        emit_block(g, hh, pp)
                        for fn in after[(hh, g, pp)]:
                            fn()

    if not nc.is_finalized():
        nc.finalize()
    return nc


_NC_CACHE = None


def _get_program():
    global _NC_CACHE
    if _NC_CACHE is None:
        _NC_CACHE = build_program()
    return _NC_CACHE


def kernel(x, Wqkv, Wout, bout, _trace=False, _trace_kwargs=None):
    x = np.asarray(x, dtype=np.float32)
    Wqkv = np.asarray(Wqkv, dtype=np.float32)
    Wout = np.asarray(Wout, dtype=np.float32)
    bout = np.asarray(bout, dtype=np.float32)

    bf = ml_dtypes.bfloat16
    scale = HEAD_DIM ** -0.5
    wq = np.ascontiguousarray((Wqkv[:, 0:DIM] * scale).astype(bf))
    wk = np.ascontiguousarray(Wqkv[:, DIM:2 * DIM].astype(bf))
    wv = np.ascontiguousarray(Wqkv[:, 2 * DIM:3 * DIM].astype(bf))
    wout_bf = np.ascontiguousarray(Wout.astype(bf))
    x_bf = x.astype(bf)

    in_maps = []
    for c in range(NCORES):
        bi, u = c // 2, c % 2
        in_maps.append({
            "xq": np.ascontiguousarray(x_bf[bi, u * NQ:(u + 1) * NQ, :]),
            "xkv": np.ascontiguousarray(x_bf[bi]),
            "wq": wq, "wk": wk, "wv": wv,
            "wout": wout_bf,
            "bout": bout,
        })

    nc = _get_program()
    kwargs = {}
    if _trace:
        kwargs["trace"] = True
        if _trace_kwargs:
            kwargs.update(_trace_kwargs)
    res = run_bass_kernel_spmd(nc, in_maps, core_ids=list(range(NCORES)), **kwargs)

    outf = np.empty((B, N, DIM), dtype=np.float32)
    for c in range(NCORES):
        bi, u = c // 2, c % 2
        outf[bi, u * NQ:(u + 1) * NQ, :] = res.results[c]["out"]
    if _trace:
        return outf, res
    return outf


# revision 25
# speedup vs baseline: 2.4300x; 1.0661x over previous
"""KeOps-style multi-head attention (unnormalized-exp softmax) on 8 trn2 cores.

Sharding: core c handles batch bi = c//2 and query rows u*1024..(u+1)*1024
(u = c%2), ALL 8 heads. Output is a pure concat over cores (no reduction).

Engine budget (per core, under the sustained-load PE clock throttle to
~1.2 GHz): the ACT engine's 128 exp instructions ([128,1024] each,
~1.2us) are the ~154us spine; everything else is arranged to hide under
it. Scores matmuls run in bf16 (1 col/cycle, row-tiled pairs overlap),
the numerator runs as fp8e4 DoubleRow over 256-key pairs (2 MACs/cycle),
and exp is written as fp8 e' = exp(s - CBIAS) (the softmax ratio is
exactly invariant to the shift; CBIAS keeps e' inside fp8e4's max 240).

Program order matters because engines execute in-order: the attention
blocks are emitted as soon as kT[0]/v2/qT[0] exist, and the remaining
projections, the per-(g,hh) denom->recip->normalize chains, and the
output projection are interleaved between blocks so they overlap the
ACT-bound loop. x arrives pre-transposed via DMA-transpose loads.
"""

import numpy as np
import ml_dtypes
from contextlib import ExitStack

import concourse.bass as bass
import concourse.mybir as mybir
import concourse.tile as tile
from concourse import bacc
from concourse.bass_utils import run_bass_kernel_spmd

DIM = 256
NUM_HEADS = 8
HEAD_DIM = 32
B = 4
N = 2048
NQ = 1024          # query rows per core
NCORES = 8
FP = mybir.dt.float32
BF = mybir.dt.bfloat16
FP8 = mybir.dt.float8e4
EXP = mybir.ActivationFunctionType.Exp
CBIAS = 3.8   # exp(s - CBIAS): this stack's float8e4 is IEEE e4m3 (max finite
              # 240); smax~8.6 plus bf16 jitter -> e^4.9=134 < 240

NT_KV = N // 128   # 16 n-tiles of kv rows
NGQ = NQ // 512    # 2 groups of 512 query cols in q^T
NGK = N // 512     # 4 groups in k^T free dim
NJ = N // 128      # 16 key chunks of 128
NT_Q = NQ // 128   # 8 output row tiles


def build_program():
    nc = bacc.Bacc()

    xq = nc.declare_dram_parameter("xq", [NQ, DIM], BF, isOutput=False)
    xkv = nc.declare_dram_parameter("xkv", [N, DIM], BF, isOutput=False)
    wq = nc.declare_dram_parameter("wq", [DIM, DIM], BF, isOutput=False)
    wk = nc.declare_dram_parameter("wk", [DIM, DIM], BF, isOutput=False)
    wv = nc.declare_dram_parameter("wv", [DIM, DIM], BF, isOutput=False)
    wout = nc.declare_dram_parameter("wout", [DIM, DIM], BF, isOutput=False)
    bout = nc.declare_dram_parameter("bout", [DIM], FP, isOutput=False)
    out = nc.declare_dram_parameter("out", [NQ, DIM], FP, isOutput=True)

    with tile.TileContext(nc) as tc, ExitStack() as ctx:
        consts = ctx.enter_context(tc.tile_pool(name="consts", bufs=1))
        persist = ctx.enter_context(tc.tile_pool(name="persist", bufs=1))

        # ---- transposed x via DMA-transpose (no PE transposes needed) ----
        # xkvT[c, ck, n] = xkv[n, 128*ck + c]; loaded in 512-row chunks so
        # the K/V projections can start before the whole tensor lands.
        xkvT = persist.tile([128, 2, N], BF)
        xqT = persist.tile([128, 2, NQ], BF)
        order = [("kv", 0), ("kv", 1), ("q", 0), ("kv", 2), ("kv", 3), ("q", 1)]
        for kind, c in order:
            if kind == "kv":
                nc.sync.dma_start_transpose(
                    out=xkvT[:, :, 512 * c:512 * (c + 1)],
                    in_=xkv[512 * c:512 * (c + 1), :])
            else:
                nc.sync.dma_start_transpose(
                    out=xqT[:, :, 512 * c:512 * (c + 1)],
                    in_=xq[512 * c:512 * (c + 1), :])

        # ---- weights / consts ----
        wq_sb = consts.tile([128, 2, DIM], BF)
        wk_sb = consts.tile([128, 2, DIM], BF)
        wv_sb = consts.tile([128, 2, DIM], BF)
        wout_sb = consts.tile([128, 2, DIM], BF)
        for ck in range(2):
            nc.sync.dma_start(out=wk_sb[:, ck, :], in_=wk[128 * ck:128 * (ck + 1), :])
            nc.sync.dma_start(out=wq_sb[:, ck, :], in_=wq[128 * ck:128 * (ck + 1), :])
        for ck in range(2):
            nc.sync.dma_start(out=wv_sb[:, ck, :], in_=wv[128 * ck:128 * (ck + 1), :])
            nc.sync.dma_start(out=wout_sb[:, ck, :], in_=wout[128 * ck:128 * (ck + 1), :])
        negC = consts.tile([128, 1], FP)
        nc.vector.memset(negC, -CBIAS)
        bias_b = consts.tile([128, DIM], FP)
        nc.sync.dma_start(out=bias_b, in_=bout[:].unsqueeze(0).to_broadcast([128, DIM]))

        # ---- persistent attention state ----
        # qT/kT stacked-head layout: tensor i holds heads 4i..4i+3; row
        # 32*hloc + d <-> head 4i+hloc, dim d.
        qT = [persist.tile([128, NQ], BF, tag=f"qT{i}", name=f"qT{i}") for i in range(2)]
        kT = [persist.tile([128, N], BF, tag=f"kT{i}", name=f"kT{i}") for i in range(2)]
        # v fp8 layout for DoubleRow: [128(n), t(key-pair), s(chunk), h, 36];
        # col 32 is the ones column (denominator); 36-wide slots keep the
        # DoubleRow pair stride 16B-aligned (8*36=288).
        v2 = persist.tile([128, NJ // 2, 2, NUM_HEADS, 36], FP8)
        nc.vector.memset(v2[:, :, :, :, HEAD_DIM:HEAD_DIM + 1], 1.0)
        PTraw = [persist.tile([128, NQ], FP, tag=f"PTr{i}", name=f"PTr{i}") for i in range(2)]
        PTb = [persist.tile([128, NQ], BF, tag=f"PTb{i}", name=f"PTb{i}") for i in range(2)]
        rb = [persist.tile([128, NQ], FP, tag=f"rb{i}", name=f"rb{i}") for i in range(2)]
        # (g,hh) group gi=2g+hh lives at partition 32*gi (+hloc) so DVE ops
        # on a group's 4 rows start at a 32-aligned partition base
        denom = persist.tile([128, 512], FP)
        recip = persist.tile([128, 512], FP)
        # out-projection partials: obp[t] = PTb[0].T @ Wout[0:128] + bias,
        # accumulated early so the tail only runs the i=1 halves
        obp = [persist.tile([128, DIM], FP, tag=f"obp{t}", name=f"obp{t}")
               for t in range(NT_Q)]

        with (
            tc.tile_pool(name="spsum", bufs=3, space="PSUM") as spsum,
            tc.tile_pool(name="npsum", bufs=1, space="PSUM") as npsum,
            tc.tile_pool(name="esb", bufs=4) as esb,
            tc.tile_pool(name="evac", bufs=4) as evac,
            tc.tile_pool(name="osb", bufs=4) as osb,
            tc.tile_pool(name="dscratch", bufs=1, space="DRAM") as dsc,
        ):
            recip_dram = dsc.tile([16, 512], FP)

            def emit_qproj(i, g):
                ps = spsum.tile([128, 1024], FP, tag="sp", name="pjq")
                for ck in range(2):
                    nc.tensor.matmul(
                        ps[:, 0:512], lhsT=wq_sb[:, ck, 128 * i:128 * (i + 1)],
                        rhs=xqT[:, ck, 512 * g:512 * (g + 1)],
                        start=(ck == 0), stop=(ck == 1))
                nc.vector.tensor_copy(qT[i][:, 512 * g:512 * (g + 1)], ps[:, 0:512])

            def emit_kproj(i, gs=None):
                for g in (range(NGK) if gs is None else gs):
                    ps = spsum.tile([128, 1024], FP, tag="sp", name="pjk")
                    for ck in range(2):
                        nc.tensor.matmul(
                            ps[:, 0:512], lhsT=wk_sb[:, ck, 128 * i:128 * (i + 1)],
                            rhs=xkvT[:, ck, 512 * g:512 * (g + 1)],
                            start=(ck == 0), stop=(ck == 1))
                    nc.vector.tensor_copy(kT[i][:, 512 * g:512 * (g + 1)], ps[:, 0:512])

            def emit_vproj(ts):
                for t in ts:
                    ps = spsum.tile([128, 1024], FP, tag="sp", name="pjv")
                    for ck in range(2):
                        nc.tensor.matmul(
                            ps[:, 0:DIM], lhsT=xkvT[:, ck, 128 * t:128 * (t + 1)],
                            rhs=wv_sb[:, ck, :],
                            start=(ck == 0), stop=(ck == 1))
                    nc.vector.tensor_copy(v2[:, t // 2, t % 2, :, 0:HEAD_DIM],
                                          ps[:, 0:DIM])

            def emit_block(g, hh, pp, hooks=None):
                nps = [npsum.tile([HEAD_DIM + 1, 512], FP, tag=f"np{x}",
                                  name=f"np{x}") for x in range(2)]
                e2 = None
                for j in range(NJ):
                    if hooks and j in hooks:
                        for fn in hooks[j]:
                            fn()
                    sp = spsum.tile([128, 1024], FP, tag="sp", name="sp")
                    for uu in range(2):
                        hloc = 2 * pp + uu
                        r = 32 * hloc
                        nc.tensor.matmul(
                            sp[:, 512 * uu:512 * (uu + 1)],
                            lhsT=kT[hh][r:r + 32, 128 * j:128 * (j + 1)],
                            rhs=qT[hh][r:r + 32, 512 * g:512 * (g + 1)],
                            start=True, stop=True,
                            tile_position=(r, 0))
                    if j % 2 == 0:
                        e2 = esb.tile([128, 2, 1024], FP8, tag="e2", name="e2")
                    nc.scalar.activation(e2[:, j % 2, :], sp, EXP, bias=negC[:, 0:1])
                    if j % 2 == 1:
                        t = j // 2
                        for uu in range(2):
                            h = 4 * hh + 2 * pp + uu
                            nc.tensor.matmul(
                                nps[uu],
                                lhsT=v2[:, t, 0:2, h, 0:HEAD_DIM + 1],
                                rhs=e2[:, 0:2, 512 * uu:512 * (uu + 1)],
                                start=(t == 0), stop=(t == NJ // 2 - 1),
                                perf_mode=mybir.MatmulPerfMode.DoubleRow)
                for uu in range(2):
                    hloc = 2 * pp + uu
                    tmp = evac.tile([HEAD_DIM + 1, 512], FP, tag="ev", name="ev")
                    nc.vector.tensor_copy(tmp, nps[uu])
                    nc.sync.dma_start(
                        out=PTraw[hh][32 * hloc:32 * hloc + 32,
                                      512 * g:512 * (g + 1)],
                        in_=tmp[0:HEAD_DIM, :])
                    r = 32 * (2 * g + hh) + hloc
                    nc.sync.dma_start(out=denom[r:r + 1, :],
                                      in_=tmp[HEAD_DIM:HEAD_DIM + 1, :])

            def emit_norm(g, hh):
                # denom -> recip -> DRAM-bounce partition-broadcast ->
                # normalized bf16 PT for this (g, hh); runs off the PE/ACT
                r0 = 32 * (2 * g + hh)
                d0 = 4 * (2 * g + hh)
                # eps (1e-6) skipped: denom' = e^-C * sum(e^s) is ~1e2 here, so
                # the reference's +1e-6 changes nothing at fp32 resolution
                nc.vector.reciprocal(recip[r0:r0 + 4, :], denom[r0:r0 + 4, :])
                nc.sync.dma_start(out=recip_dram[d0:d0 + 4, :],
                                  in_=recip[r0:r0 + 4, :])
                for hloc in range(4):
                    nc.sync.dma_start(
                        out=rb[hh][32 * hloc:32 * hloc + 32,
                                   512 * g:512 * (g + 1)],
                        in_=recip_dram[d0 + hloc:d0 + hloc + 1, :].to_broadcast([32, 512]))
                nc.vector.tensor_mul(PTb[hh][:, 512 * g:512 * (g + 1)],
                                     PTraw[hh][:, 512 * g:512 * (g + 1)],
                                     rb[hh][:, 512 * g:512 * (g + 1)])

            def emit_oph(t, i):
                # one half of the output projection for row-tile t; i=0
                # stashes partial+bias in obp[t], i=1 completes and stores
                ps = spsum.tile([128, 1024], FP, tag="sp", name="spo")
                nc.tensor.matmul(
                    ps[:, 0:DIM], lhsT=PTb[i][:, 128 * t:128 * (t + 1)],
                    rhs=wout_sb[:, i, :], start=True, stop=True)
                if i == 0:
                    nc.vector.tensor_add(obp[t], ps[:, 0:DIM], bias_b)
                else:
                    ob = osb.tile([128, DIM], FP, tag="ob", name="ob")
                    nc.vector.tensor_add(ob, ps[:, 0:DIM], obp[t])
                    nc.sync.dma_start(out=out[128 * t:128 * (t + 1), :], in_=ob)

            # ---- minimal pre-loop projections ----
            emit_kproj(0)
            emit_qproj(0, 0)
            emit_vproj(range(NT_KV))

            # ---- attention blocks; leftover projections, normalize
            # chains, and out-projection halves sit between blocks where
            # their inputs are long since ready ----
            after = {
                (0, 0, 0): [lambda: emit_qproj(0, 1)],
                (0, 0, 1): [lambda: emit_kproj(1),
                            lambda: emit_norm(0, 0)],
                (0, 1, 0): [lambda: emit_qproj(1, 0)],
                (0, 1, 1): [lambda: emit_qproj(1, 1),
                            lambda: emit_norm(1, 0)],
                (1, 0, 0): [lambda t=t: emit_oph(t, 0) for t in range(0, 4)],
                (1, 0, 1): [lambda: emit_norm(0, 1)] +
                           [lambda t=t: emit_oph(t, 0) for t in range(4, NT_Q)],
                (1, 1, 0): [lambda t=t: emit_oph(t, 1) for t in range(0, 4)],
                (1, 1, 1): [lambda: emit_norm(1, 1)] +
                           [lambda t=t: emit_oph(t, 1) for t in range(4, NT_Q)],
            }
            for hh in range(2):
                for g in range(NGQ):
                    for pp in range(2):
                        emit_block(g, hh, pp)
                        for fn in after[(hh, g, pp)]:
                            fn()

    if not nc.is_finalized():
        nc.finalize()
    return nc


_NC_CACHE = None


def _get_program():
    global _NC_CACHE
    if _NC_CACHE is None:
        _NC_CACHE = build_program()
    return _NC_CACHE


def kernel(x, Wqkv, Wout, bout, _trace=False, _trace_kwargs=None):
    x = np.asarray(x, dtype=np.float32)
    Wqkv = np.asarray(Wqkv, dtype=np.float32)
    Wout = np.asarray(Wout, dtype=np.float32)
    bout = np.asarray(bout, dtype=np.float32)

    bf = ml_dtypes.bfloat16
    scale = HEAD_DIM ** -0.5
    wq = np.ascontiguousarray((Wqkv[:, 0:DIM] * scale).astype(bf))
    wk = np.ascontiguousarray(Wqkv[:, DIM:2 * DIM].astype(bf))
    wv = np.ascontiguousarray(Wqkv[:, 2 * DIM:3 * DIM].astype(bf))
    wout_bf = np.ascontiguousarray(Wout.astype(bf))
    x_bf = x.astype(bf)

    in_maps = []
    for c in range(NCORES):
        bi, u = c // 2, c % 2
        in_maps.append({
            "xq": np.ascontiguousarray(x_bf[bi, u * NQ:(u + 1) * NQ, :]),
            "xkv": np.ascontiguousarray(x_bf[bi]),
            "wq": wq, "wk": wk, "wv": wv,
            "wout": wout_bf,
            "bout": bout,
        })

    nc = _get_program()
    kwargs = {}
    if _trace:
        kwargs["trace"] = True
        if _trace_kwargs:
            kwargs.update(_trace_kwargs)
    res = run_bass_kernel_spmd(nc, in_maps, core_ids=list(range(NCORES)), **kwargs)

    outf = np.empty((B, N, DIM), dtype=np.float32)
    for c in range(NCORES):
        bi, u = c // 2, c % 2
        outf[bi, u * NQ:(u + 1) * NQ, :] = res.results[c]["out"]
    if _trace:
        return outf, res
    return outf


# revision 26
# speedup vs baseline: 2.4952x; 1.0268x over previous
"""KeOps-style multi-head attention (unnormalized-exp softmax) on 8 trn2 cores.

Sharding: core c handles batch bi = c//2 and query rows u*1024..(u+1)*1024
(u = c%2), ALL 8 heads. Output is a pure concat over cores (no reduction).

Engine budget (per core, under the sustained-load PE clock throttle to
~1.2 GHz): the ACT engine's 128 exp instructions ([128,1024] each,
~1.2us) are the ~154us spine; everything else is arranged to hide under
it. Scores matmuls run in bf16 (1 col/cycle, row-tiled pairs overlap),
the numerator runs as fp8e4 DoubleRow over 256-key pairs (2 MACs/cycle),
and exp is written as fp8 e' = exp(s - CBIAS) (the softmax ratio is
exactly invariant to the shift; CBIAS keeps e' inside fp8e4's max 240).

Program order matters because engines execute in-order: the attention
blocks are emitted as soon as kT[0]/v2/qT[0] exist, and the remaining
projections, the per-(g,hh) denom->recip->normalize chains, and the
output projection are interleaved between blocks so they overlap the
ACT-bound loop. x arrives pre-transposed via DMA-transpose loads.
"""

import numpy as np
import ml_dtypes
from contextlib import ExitStack

import concourse.bass as bass
import concourse.mybir as mybir
import concourse.tile as tile
from concourse import bacc
from concourse.bass_utils import run_bass_kernel_spmd

DIM = 256
NUM_HEADS = 8
HEAD_DIM = 32
B = 4
N = 2048
NQ = 1024          # query rows per core
NCORES = 8
FP = mybir.dt.float32
BF = mybir.dt.bfloat16
FP8 = mybir.dt.float8e4
EXP = mybir.ActivationFunctionType.Exp
CBIAS = 3.8   # exp(s - CBIAS): this stack's float8e4 is IEEE e4m3 (max finite
              # 240); smax~8.6 plus bf16 jitter -> e^4.9=134 < 240

NT_KV = N // 128   # 16 n-tiles of kv rows
NGQ = NQ // 512    # 2 groups of 512 query cols in q^T
NGK = N // 512     # 4 groups in k^T free dim
NJ = N // 128      # 16 key chunks of 128
NT_Q = NQ // 128   # 8 output row tiles


def build_program():
    nc = bacc.Bacc()

    xq = nc.declare_dram_parameter("xq", [NQ, DIM], BF, isOutput=False)
    xkv = nc.declare_dram_parameter("xkv", [N, DIM], BF, isOutput=False)
    wq = nc.declare_dram_parameter("wq", [DIM, DIM], BF, isOutput=False)
    wk = nc.declare_dram_parameter("wk", [DIM, DIM], BF, isOutput=False)
    wv = nc.declare_dram_parameter("wv", [DIM, DIM], BF, isOutput=False)
    wout = nc.declare_dram_parameter("wout", [DIM, DIM], BF, isOutput=False)
    bout = nc.declare_dram_parameter("bout", [DIM], FP, isOutput=False)
    out = nc.declare_dram_parameter("out", [NQ, DIM], FP, isOutput=True)

    with tile.TileContext(nc) as tc, ExitStack() as ctx:
        consts = ctx.enter_context(tc.tile_pool(name="consts", bufs=1))
        persist = ctx.enter_context(tc.tile_pool(name="persist", bufs=1))

        # ---- transposed x via DMA-transpose (no PE transposes needed) ----
        # xkvT[c, ck, n] = xkv[n, 128*ck + c]; loaded in 512-row chunks so
        # the K/V projections can start before the whole tensor lands.
        xkvT = persist.tile([128, 2, N], BF)
        xqT = persist.tile([128, 2, NQ], BF)
        order = [("kv", 0), ("kv", 1), ("q", 0), ("kv", 2), ("kv", 3), ("q", 1)]
        for kind, c in order:
            if kind == "kv":
                nc.sync.dma_start_transpose(
                    out=xkvT[:, :, 512 * c:512 * (c + 1)],
                    in_=xkv[512 * c:512 * (c + 1), :])
            else:
                nc.sync.dma_start_transpose(
                    out=xqT[:, :, 512 * c:512 * (c + 1)],
                    in_=xq[512 * c:512 * (c + 1), :])

        # ---- weights / consts ----
        wq_sb = consts.tile([128, 2, DIM], BF)
        wk_sb = consts.tile([128, 2, DIM], BF)
        wv_sb = consts.tile([128, 2, DIM], BF)
        wout_sb = consts.tile([128, 2, DIM], BF)
        for ck in range(2):
            nc.sync.dma_start(out=wk_sb[:, ck, :], in_=wk[128 * ck:128 * (ck + 1), :])
            nc.sync.dma_start(out=wq_sb[:, ck, :], in_=wq[128 * ck:128 * (ck + 1), :])
        for ck in range(2):
            nc.sync.dma_start(out=wv_sb[:, ck, :], in_=wv[128 * ck:128 * (ck + 1), :])
            nc.sync.dma_start(out=wout_sb[:, ck, :], in_=wout[128 * ck:128 * (ck + 1), :])
        negC = consts.tile([128, 1], FP)
        nc.vector.memset(negC, -CBIAS)
        bias_b = consts.tile([128, DIM], FP)
        nc.sync.dma_start(out=bias_b, in_=bout[:].unsqueeze(0).to_broadcast([128, DIM]))

        # ---- persistent attention state ----
        # qT/kT stacked-head layout: tensor i holds heads 4i..4i+3; row
        # 32*hloc + d <-> head 4i+hloc, dim d.
        qT = [persist.tile([128, NQ], BF, tag=f"qT{i}", name=f"qT{i}") for i in range(2)]
        kT = [persist.tile([128, N], BF, tag=f"kT{i}", name=f"kT{i}") for i in range(2)]
        # v fp8 layout for DoubleRow: [128(n), t(key-pair), s(chunk), h, 36];
        # col 32 is the ones column (denominator); 36-wide slots keep the
        # DoubleRow pair stride 16B-aligned (8*36=288).
        v2 = persist.tile([128, NJ // 2, 2, NUM_HEADS, 36], FP8)
        nc.vector.memset(v2[:, :, :, :, HEAD_DIM:HEAD_DIM + 1], 1.0)
        PTraw = [persist.tile([128, NQ], FP, tag=f"PTr{i}", name=f"PTr{i}") for i in range(2)]
        PTb = [persist.tile([128, NQ], BF, tag=f"PTb{i}", name=f"PTb{i}") for i in range(2)]
        rb = [persist.tile([128, NQ], FP, tag=f"rb{i}", name=f"rb{i}") for i in range(2)]
        # (g,hh) group gi=2g+hh lives at partition 32*gi (+hloc) so DVE ops
        # on a group's 4 rows start at a 32-aligned partition base
        denom = persist.tile([128, 512], FP)
        recip = persist.tile([128, 512], FP)
        # out-projection partials: obp[t] = PTb[0].T @ Wout[0:128] + bias,
        # accumulated early so the tail only runs the i=1 halves
        obp = [persist.tile([128, DIM], FP, tag=f"obp{t}", name=f"obp{t}")
               for t in range(NT_Q)]

        with (
            tc.tile_pool(name="spsum", bufs=3, space="PSUM") as spsum,
            tc.tile_pool(name="npsum", bufs=1, space="PSUM") as npsum,
            tc.tile_pool(name="esb", bufs=4) as esb,
            tc.tile_pool(name="evac", bufs=4) as evac,
            tc.tile_pool(name="osb", bufs=4) as osb,
            tc.tile_pool(name="dscratch", bufs=1, space="DRAM") as dsc,
        ):
            recip_dram = dsc.tile([16, 512], FP)

            def emit_qproj(i, g):
                ps = spsum.tile([128, 1024], FP, tag="sp", name="pjq")
                for ck in range(2):
                    nc.tensor.matmul(
                        ps[:, 0:512], lhsT=wq_sb[:, ck, 128 * i:128 * (i + 1)],
                        rhs=xqT[:, ck, 512 * g:512 * (g + 1)],
                        start=(ck == 0), stop=(ck == 1))
                nc.vector.tensor_copy(qT[i][:, 512 * g:512 * (g + 1)], ps[:, 0:512])

            def emit_kproj(i, gs=None):
                gl = list(range(NGK) if gs is None else gs)
                for g0 in gl[::2]:
                    ps = spsum.tile([128, 1024], FP, tag="sp", name="pjk")
                    for o, g in enumerate((g0, g0 + 1)):
                        for ck in range(2):
                            nc.tensor.matmul(
                                ps[:, 512 * o:512 * (o + 1)],
                                lhsT=wk_sb[:, ck, 128 * i:128 * (i + 1)],
                                rhs=xkvT[:, ck, 512 * g:512 * (g + 1)],
                                start=(ck == 0), stop=(ck == 1))
                    for o, g in enumerate((g0, g0 + 1)):
                        nc.vector.tensor_copy(kT[i][:, 512 * g:512 * (g + 1)],
                                              ps[:, 512 * o:512 * (o + 1)])

            def emit_vproj(ts):
                for t in ts:
                    ps = spsum.tile([128, 1024], FP, tag="sp", name="pjv")
                    for ck in range(2):
                        nc.tensor.matmul(
                            ps[:, 0:DIM], lhsT=xkvT[:, ck, 128 * t:128 * (t + 1)],
                            rhs=wv_sb[:, ck, :],
                            start=(ck == 0), stop=(ck == 1))
                    nc.vector.tensor_copy(v2[:, t // 2, t % 2, :, 0:HEAD_DIM],
                                          ps[:, 0:DIM])

            def emit_block(g, hh, pp, hooks=None):
                nps = [npsum.tile([HEAD_DIM + 1, 512], FP, tag=f"np{x}",
                                  name=f"np{x}") for x in range(2)]
                e2 = None
                for j in range(NJ):
                    if hooks and j in hooks:
                        for fn in hooks[j]:
                            fn()
                    sp = spsum.tile([128, 1024], FP, tag="sp", name="sp")
                    for uu in range(2):
                        hloc = 2 * pp + uu
                        r = 32 * hloc
                        nc.tensor.matmul(
                            sp[:, 512 * uu:512 * (uu + 1)],
                            lhsT=kT[hh][r:r + 32, 128 * j:128 * (j + 1)],
                            rhs=qT[hh][r:r + 32, 512 * g:512 * (g + 1)],
                            start=True, stop=True,
                            tile_position=(r, 0))
                    if j % 2 == 0:
                        e2 = esb.tile([128, 2, 1024], FP8, tag="e2", name="e2")
                    nc.scalar.activation(e2[:, j % 2, :], sp, EXP, bias=negC[:, 0:1])
                    if j % 2 == 1:
                        t = j // 2
                        for uu in range(2):
                            h = 4 * hh + 2 * pp + uu
                            nc.tensor.matmul(
                                nps[uu],
                                lhsT=v2[:, t, 0:2, h, 0:HEAD_DIM + 1],
                                rhs=e2[:, 0:2, 512 * uu:512 * (uu + 1)],
                                start=(t == 0), stop=(t == NJ // 2 - 1),
                                perf_mode=mybir.MatmulPerfMode.DoubleRow)
                for uu in range(2):
                    hloc = 2 * pp + uu
                    tmp = evac.tile([HEAD_DIM + 1, 512], FP, tag="ev", name="ev")
                    nc.vector.tensor_copy(tmp, nps[uu])
                    nc.sync.dma_start(
                        out=PTraw[hh][32 * hloc:32 * hloc + 32,
                                      512 * g:512 * (g + 1)],
                        in_=tmp[0:HEAD_DIM, :])
                    r = 32 * (2 * g + hh) + hloc
                    nc.sync.dma_start(out=denom[r:r + 1, :],
                                      in_=tmp[HEAD_DIM:HEAD_DIM + 1, :])

            def emit_norm(g, hh):
                # denom -> recip -> DRAM-bounce partition-broadcast ->
                # normalized bf16 PT for this (g, hh); runs off the PE/ACT
                r0 = 32 * (2 * g + hh)
                d0 = 4 * (2 * g + hh)
                # eps (1e-6) skipped: denom' = e^-C * sum(e^s) is ~1e2 here, so
                # the reference's +1e-6 changes nothing at fp32 resolution
                nc.vector.reciprocal(recip[r0:r0 + 4, :], denom[r0:r0 + 4, :])
                nc.sync.dma_start(out=recip_dram[d0:d0 + 4, :],
                                  in_=recip[r0:r0 + 4, :])
                for hloc in range(4):
                    nc.sync.dma_start(
                        out=rb[hh][32 * hloc:32 * hloc + 32,
                                   512 * g:512 * (g + 1)],
                        in_=recip_dram[d0 + hloc:d0 + hloc + 1, :].to_broadcast([32, 512]))
                nc.vector.tensor_mul(PTb[hh][:, 512 * g:512 * (g + 1)],
                                     PTraw[hh][:, 512 * g:512 * (g + 1)],
                                     rb[hh][:, 512 * g:512 * (g + 1)])

            def emit_oph(ts, i):
                # one half of the output projection for row-tiles ts (<=4,
                # sharing one psum tile); i=0 stashes partial+bias in obp[t],
                # i=1 completes and stores
                ps = spsum.tile([128, 1024], FP, tag="sp", name="spo")
                for o, t in enumerate(ts):
                    nc.tensor.matmul(
                        ps[:, DIM * o:DIM * (o + 1)],
                        lhsT=PTb[i][:, 128 * t:128 * (t + 1)],
                        rhs=wout_sb[:, i, :], start=True, stop=True)
                for o, t in enumerate(ts):
                    if i == 0:
                        nc.vector.tensor_add(obp[t], ps[:, DIM * o:DIM * (o + 1)], bias_b)
                    else:
                        ob = osb.tile([128, DIM], FP, tag="ob", name=f"ob{o}")
                        nc.vector.tensor_add(ob, ps[:, DIM * o:DIM * (o + 1)], obp[t])
                        nc.sync.dma_start(out=out[128 * t:128 * (t + 1), :], in_=ob)

            # ---- minimal pre-loop projections ----
            emit_kproj(0)
            emit_qproj(0, 0)
            emit_vproj(range(NT_KV))

            # ---- attention blocks; leftover projections, normalize
            # chains, and out-projection halves sit between blocks where
            # their inputs are long since ready ----
            after = {
                (0, 0, 0): [lambda: emit_qproj(0, 1)],
                (0, 0, 1): [lambda: emit_kproj(1),
                            lambda: emit_norm(0, 0)],
                (0, 1, 0): [lambda: emit_qproj(1, 0)],
                (0, 1, 1): [lambda: emit_qproj(1, 1),
                            lambda: emit_norm(1, 0)],
                (1, 0, 0): [lambda: emit_oph(range(0, 4), 0)],
                (1, 0, 1): [lambda: emit_norm(0, 1),
                            lambda: emit_oph(range(4, NT_Q), 0)],
                (1, 1, 0): [lambda: emit_oph(range(0, 4), 1)],
                (1, 1, 1): [lambda: emit_norm(1, 1),
                            lambda: emit_oph(range(4, NT_Q), 1)],
            }
            for hh in range(2):
                for g in range(NGQ):
                    for pp in range(2):
                        emit_block(g, hh, pp)
                        for fn in after[(hh, g, pp)]:
                            fn()

    if not nc.is_finalized():
        nc.finalize()
    return nc


_NC_CACHE = None


def _get_program():
    global _NC_CACHE
    if _NC_CACHE is None:
        _NC_CACHE = build_program()
    return _NC_CACHE


def kernel(x, Wqkv, Wout, bout, _trace=False, _trace_kwargs=None):
    x = np.asarray(x, dtype=np.float32)
    Wqkv = np.asarray(Wqkv, dtype=np.float32)
    Wout = np.asarray(Wout, dtype=np.float32)
    bout = np.asarray(bout, dtype=np.float32)

    bf = ml_dtypes.bfloat16
    scale = HEAD_DIM ** -0.5
    wq = np.ascontiguousarray((Wqkv[:, 0:DIM] * scale).astype(bf))
    wk = np.ascontiguousarray(Wqkv[:, DIM:2 * DIM].astype(bf))
    wv = np.ascontiguousarray(Wqkv[:, 2 * DIM:3 * DIM].astype(bf))
    wout_bf = np.ascontiguousarray(Wout.astype(bf))
    x_bf = x.astype(bf)

    in_maps = []
    for c in range(NCORES):
        bi, u = c // 2, c % 2
        in_maps.append({
            "xq": np.ascontiguousarray(x_bf[bi, u * NQ:(u + 1) * NQ, :]),
            "xkv": np.ascontiguousarray(x_bf[bi]),
            "wq": wq, "wk": wk, "wv": wv,
            "wout": wout_bf,
            "bout": bout,
        })

    nc = _get_program()
    kwargs = {}
    if _trace:
        kwargs["trace"] = True
        if _trace_kwargs:
            kwargs.update(_trace_kwargs)
    res = run_bass_kernel_spmd(nc, in_maps, core_ids=list(range(NCORES)), **kwargs)

    outf = np.empty((B, N, DIM), dtype=np.float32)
    for c in range(NCORES):
        bi, u = c // 2, c % 2
        outf[bi, u * NQ:(u + 1) * NQ, :] = res.results[c]["out"]
    if _trace:
        return outf, res
    return outf
